# revision 27
# baseline (speedup 1.0000x reference)
"""BigBird block-sparse attention TRN2 kernel v2 (8 NeuronCores, SPMD).

Sharding: core c handles batch b=c//2 and head-half hh=c%2 (8 of 16 heads,
feature slice hh*512..+512). All matmul I/O in bf16 (fp32 PSUM accumulate).

v2 structure (vs v1): heads processed in PAIRS with the even head's K=64
matmuls on PE rows 0-63 and the odd head's on rows 64-127, emitted
adjacently so the row-tiled matmuls run concurrently (~2x on QK^T and the
edge scores). Middle blocks are processed in 30 half-strip units per pair
(2 query blocks x 2 heads), with sps PSUM laid out bank-disjoint between
the heads ([128,7,128]: even groups 0-2 in bank 0, odd 4-6 in bank 1).
exp runs as one ACT instruction over a strided 2x384 AP; sliding-window
bans are GpSimd memsets on the bf16 pt tile. Projection m-tile passes and
edge-block scores are interleaved into the unit stream as PE filler so the
PE never idles (HAM stays at K=8/8). q-scale (1/8) and q-bias are folded
into Wq/bq host-side; v carries no bias on device (host adds bv after
normalization since sum(softmax)=1).

Per core, single pass, q/k/v SBUF-resident:
  1. v-pass (streams x once): v = X@Wv.T -> via SBUF->SBUF shift DMAs into
     vsh [128, 32, 520]: 64-row-shifted key chunks (chunk c = seq 64+128c),
     chunk 31 = [block63 | block0], ones column per head (col h*65+64) for
     softmax denominators.
  2. k/q m-tile passes: k.T, q.T feature-major [128, 4096] bf16 tiles,
     biases via DVE tensor_scalar.
  3. per pair: 30 half-units (QK^T paired-row matmuls -> exp -> ban ->
     PV vs vsh chunks + denominator rider), edge blocks 0/63 raw scores
     vs all keys and 1/62 vs 6 key blocks shipped to host.
Host: normalizes middle ctx (+bv), computes edge softmax+PV, reassembles.
"""
import sys

if "/opt/trn_rl_repo" not in sys.path:
    sys.path.insert(0, "/opt/trn_rl_repo")

import numpy as np
import ml_dtypes

import concourse.bacc as bacc
import concourse.bass as bass
import concourse.tile as tile
from concourse import mybir
from concourse.bass_utils import run_bass_kernel_spmd

F32 = mybir.dt.float32
BF16 = mybir.dt.bfloat16
NPBF16 = ml_dtypes.bfloat16

B, S, H, HS, D, BLK = 4, 4096, 16, 1024, 64, 64
NB = S // BLK            # 64 key/query blocks
HPC = 8                  # heads per core
FPC = HPC * D            # 512 features per core
NKC = HS // 128          # 8 contraction chunks
NSEQ = 8                 # seq chunks of 512
NU = 30                  # half-strip units per pair (q blocks 2..61)
GC = 31                  # [blk63|blk0] global chunk slot in vsh

_BUILT = None


def _build():
    nc = bacc.Bacc(None, target_bir_lowering=False)

    # ---- parameters ----
    # xt[p, n, kc, s'] = X[n*512+s', kc*128+p]
    xt = nc.declare_dram_parameter("xt", [128, NSEQ, NKC, 512], BF16, False)
    # w*[p, kc, f] = W.T[kc*128+p, f]  (feature slice; wq pre-scaled by 1/8)
    wq = nc.declare_dram_parameter("wq", [128, NKC, FPC], BF16, False)
    wk = nc.declare_dram_parameter("wk", [128, NKC, FPC], BF16, False)
    wv = nc.declare_dram_parameter("wv", [128, NKC, FPC], BF16, False)
    bqs = nc.declare_dram_parameter("bqs", [128, 4], F32, False)  # /8 applied
    bks = nc.declare_dram_parameter("bks", [128, 4], F32, False)

    # ctxt[pair*65+r, he*3840 + u*128 + q] : r<64 numerator, r=64 denominator
    ctxt = nc.declare_dram_parameter("ctxt", [4 * 65, 2 * NU * 128], BF16, True)
    pe1 = nc.declare_dram_parameter("pe1", [HPC * 128, S], BF16, True)
    pe2 = nc.declare_dram_parameter("pe2", [HPC * 128, 6 * BLK], BF16, True)

    with tile.TileContext(nc) as tc:
        with tc.tile_pool(name="const", bufs=1) as cp, \
             tc.tile_pool(name="big", bufs=1) as bp, \
             tc.tile_pool(name="x", bufs=1) as xp, \
             tc.tile_pool(name="evac", bufs=2) as ep, \
             tc.tile_pool(name="pair", bufs=3) as prp, \
             tc.tile_pool(name="pt", bufs=4) as ptp, \
             tc.tile_pool(name="ctx", bufs=3) as cxp, \
             tc.tile_pool(name="ee", bufs=4) as eep, \
             tc.tile_pool(name="ps1", bufs=2, space="PSUM") as pp1, \
             tc.tile_pool(name="qk", bufs=2, space="PSUM") as qkp, \
             tc.tile_pool(name="sm", bufs=2, space="PSUM") as smp:

            # ---- input DMAs, ordered so arrival matches the init phase's
            # consumption: wv gates the first matmul; xt_n stream in around
            # the per-n group rate; wk/wq needed from the first k0/q0 group
            wvt = bp.tile([128, NKC, FPC], BF16, tag="wv_kt3", name="wvt")
            xts = [xp.tile([128, NKC, 512], BF16, tag=f"xt{n}", name=f"xt{n}")
                   for n in range(NSEQ)]
            wts = {"v": wvt}
            for name, w in (("k", wk), ("q", wq)):
                wts[name] = cp.tile([128, NKC, FPC], BF16, tag=f"w{name}",
                                    name=f"w{name}t")
            bqt = cp.tile([128, 4], F32, tag="bqt")
            bkt = cp.tile([128, 4], F32, tag="bkt")
            # first v matmul gates on wv + xt0 only: spread those two across
            # all three queues ahead of everything else
            nc.scalar.dma_start(out=wvt[:, 0:4], in_=wv[:, 0:4])
            nc.gpsimd.dma_start(out=wvt[:, 4:NKC], in_=wv[:, 4:NKC])
            nc.sync.dma_start(out=xts[0][:], in_=xt[:, 0])
            nc.scalar.dma_start(out=xts[1][:], in_=xt[:, 1])
            nc.gpsimd.dma_start(out=wts["k"][:], in_=wk[:])
            nc.sync.dma_start(out=xts[2][:], in_=xt[:, 2])
            nc.scalar.dma_start(out=bkt[:], in_=bks[:])
            nc.scalar.dma_start(out=xts[3][:], in_=xt[:, 3])
            nc.gpsimd.dma_start(out=wts["q"][:], in_=wq[:])
            nc.sync.dma_start(out=bqt[:], in_=bqs[:])
            nc.sync.dma_start(out=xts[4][:], in_=xt[:, 4])
            for n, eng in ((5, nc.scalar), (6, nc.gpsimd), (7, nc.sync)):
                eng.dma_start(out=xts[n][:], in_=xt[:, n])

            # vsh: shifted v chunks + ones cols. [128, 32, 520] bf16
            vsh = bp.tile([128, 32, 520], BF16, tag="vsh")
            ones_base = vsh[:, :, 0:1]
            nc.vector.memset(
                bass.AP(tensor=ones_base.tensor, offset=ones_base.offset + 64,
                        ap=[ones_base.ap[0], [520, 32], [65, HPC], [1, 1]]),
                1.0,
            )

            # ---- v-pass ----
            ev4 = None

            def shift_dma(pdst, psrc, c0, sm0, nch):
                dst = vsh[pdst:pdst + 64, c0, 0:64]
                src = ev4[psrc:psrc + 64, sm0, 0:64]
                nc.scalar.dma_start(
                    out=bass.AP(tensor=dst.tensor, offset=dst.offset,
                                ap=[dst.ap[0], [520, nch], [65, HPC], [1, 64]]),
                    in_=bass.AP(tensor=src.tensor, offset=src.offset,
                                ap=[src.ap[0], [512, nch], [64, HPC], [1, 64]]),
                )

            def v_group(g):
                nonlocal ev4
                n, half = g // 2, g % 2
                ev4 = ep.tile([128, 2, 512], BF16, tag="ev4", name="ev4")
                for sm2 in range(2):
                    sm = 2 * half + sm2
                    ps = pp1.tile([128, 512], F32, tag="ps1")
                    for kc in range(NKC):
                        nc.tensor.matmul(
                            ps[:],
                            xts[n][:, kc, sm * 128:(sm + 1) * 128],
                            wts["v"][:, kc, :],
                            start=(kc == 0), stop=(kc == NKC - 1),
                        )
                    nc.vector.tensor_copy(ev4[:, sm2, :], ps[:])
                if g == 0:
                    shift_dma(64, 0, GC, 0, 1)
                    shift_dma(64, 0, 0, 1, 1)
                else:
                    shift_dma(64, 0, 2 * g - 1, 0, 2)
                shift_dma(0, 64, 2 * g, 0, 2)

            # ---- k/q m-tile pass groups (interleaved as PE filler) ----
            kts, qts = {}, {}

            def kq_group(name, store, bt, mt, n):
                if mt not in store:
                    tag = "wv_kt3" if (name == "k" and mt == 3) else \
                        f"{name}t{mt}"
                    store[mt] = bp.tile([128, S], BF16, tag=tag,
                                        name=f"{name}t{mt}")
                dst = store[mt]
                ps = pp1.tile([128, 512], F32, tag="ps1", name="ps")
                for kc in range(NKC):
                    nc.tensor.matmul(
                        ps[:],
                        wts[name][:, kc, mt * 128:(mt + 1) * 128],
                        xts[n][:, kc, :],
                        start=(kc == 0), stop=(kc == NKC - 1),
                    )
                nc.vector.tensor_scalar(
                    out=dst[:, n * 512:(n + 1) * 512], in0=ps[:],
                    scalar1=bt[:, mt:mt + 1], scalar2=None,
                    op0=mybir.AluOpType.add,
                )

            def kq_groups(mt):
                out = []
                for name, store, bt in (("k", kts, bkt), ("q", qts, bqt)):
                    for n in range(NSEQ):
                        out.append(lambda name=name, store=store, bt=bt,
                                   n=n: kq_group(name, store, bt, mt, n))
                return out

            # init: v-pass with k0/q0 interleaved per x-chunk so compute
            # consumption matches the xt DMA delivery rate (no PE stall)
            for n in range(NSEQ):
                v_group(2 * n)
                v_group(2 * n + 1)
                kq_group("k", kts, bkt, 0, n)
                kq_group("q", qts, bqt, 0, n)

            # ---- pair processing ----
            def mm(out, lhsT, rhs, start=True, stop=True):
                nc.tensor.matmul(out, lhsT, rhs, start=start, stop=stop)

            EXP = mybir.ActivationFunctionType.Exp

            def make_pair_tiles(pair):
                kt, qt = kts[pair], qts[pair]
                kg = prp.tile([128, 128], BF16, tag="kg")
                qec = prp.tile([128, 256], BF16, tag="qec")
                for p0 in (0, 64):
                    nc.vector.tensor_copy(kg[p0:p0 + 64, 0:64],
                                          kt[p0:p0 + 64, S - 64:S])
                    nc.vector.tensor_copy(kg[p0:p0 + 64, 64:128],
                                          kt[p0:p0 + 64, 0:64])
                    nc.vector.tensor_copy(qec[p0:p0 + 64, 0:64],
                                          qt[p0:p0 + 64, 0:64])
                    nc.vector.tensor_copy(qec[p0:p0 + 64, 64:128],
                                          qt[p0:p0 + 64, S - 64:S])
                    nc.vector.tensor_copy(qec[p0:p0 + 64, 128:192],
                                          qt[p0:p0 + 64, 64:128])
                    nc.vector.tensor_copy(qec[p0:p0 + 64, 192:256],
                                          qt[p0:p0 + 64, S - 128:S - 64])
                return kg, qec

            ctx_cur = {}

            def unit_qk(pair, u, kg):
                """Half-strip QK^T: q blocks 2+2u, 3+2u; paired rows.
                sps [128,7,128]: even head groups 0-2 (bank 0), odd 4-6
                (bank 1); adjacent e/o emission -> concurrent row tiles."""
                kt, qt = kts[pair], qts[pair]
                q0 = (2 + 2 * u) * BLK
                ca = (2 * u + 1) * BLK
                sps = qkp.tile([128, 7, 128], F32, tag="sps", name="sps")
                qa = {0: qt[0:64, q0:q0 + 128], 64: qt[64:128, q0:q0 + 128]}
                mm(sps[:, 0, :], kg[0:64, :], qa[0])
                mm(sps[:, 4, :], kg[64:128, :], qa[64])
                mm(sps[:, 1, :], kt[0:64, ca:ca + 128], qa[0])
                mm(sps[:, 5, :], kt[64:128, ca:ca + 128], qa[64])
                mm(sps[:, 2, :], kt[0:64, ca + 128:ca + 256], qa[0])
                mm(sps[:, 6, :], kt[64:128, ca + 128:ca + 256], qa[64])
                pt = ptp.tile([128, 7, 128], BF16, tag="pt", name="pt")
                # one exp over both heads' 384-col blocks via strided AP
                src = sps[:, 0, :]
                dst = pt[:, 0, :]
                nc.scalar.activation(
                    bass.AP(tensor=dst.tensor, offset=dst.offset,
                            ap=[dst.ap[0], [512, 2], [1, 384]]),
                    bass.AP(tensor=src.tensor, offset=src.offset,
                            ap=[src.ap[0], [512, 2], [1, 384]]),
                    EXP,
                )
                # ban invalid sliding quadrants: {g1,g5}, {g2,g6}
                # (GpSimd: pt is SBUF-only, frees DVE for evac copies)
                lo = pt[0:64, 1, 64:128]
                nc.gpsimd.memset(
                    bass.AP(tensor=lo.tensor, offset=lo.offset,
                            ap=[lo.ap[0], [512, 2], [1, 64]]), 0.0)
                hi = pt[64:128, 2, 0:64]
                nc.gpsimd.memset(
                    bass.AP(tensor=hi.tensor, offset=hi.offset,
                            ap=[hi.ap[0], [512, 2], [1, 64]]), 0.0)
                return pt

            def unit_pv(pair, items):
                """PV for a couple of same-pair units into one cps bank;
                one evac copy for both."""
                cps = smp.tile([65, 2, 256], F32, tag="cps", name="cps")
                for j, (u, pt) in enumerate(items):
                    for he in (0, 1):
                        hc = (2 * pair + he) * 65
                        g0 = 4 * he
                        sl = cps[:, he, j * 128:(j + 1) * 128]
                        mm(sl, vsh[:, GC, hc:hc + 65], pt[:, g0, :],
                           start=True, stop=False)
                        mm(sl, vsh[:, u, hc:hc + 65], pt[:, g0 + 1, :],
                           start=False, stop=False)
                        mm(sl, vsh[:, u + 1, hc:hc + 65], pt[:, g0 + 2, :],
                           start=False, stop=True)
                u0 = items[0][0]
                seg, off = u0 // 6, u0 % 6
                if off == 0:
                    ctx_cur[pair] = cxp.tile([65, 2, 6 * 128], BF16,
                                             tag="ctx", name="ctx")
                ctx = ctx_cur[pair]
                w = 128 * len(items)
                nc.vector.tensor_copy(ctx[:, :, off * 128:off * 128 + w],
                                      cps[:, :, 0:w])
                if off + len(items) == 6:
                    dst = ctxt[pair * 65:(pair + 1) * 65,
                               seg * 768:seg * 768 + 768]
                    nc.sync.dma_start(
                        out=bass.AP(tensor=dst.tensor, offset=dst.offset,
                                    ap=[dst.ap[0], [NU * 128, 2], [1, 768]]),
                        in_=ctx[:],
                    )

            def e1_unit(pair, c, qec):
                kt = kts[pair]
                for he, dma_eng in ((0, nc.sync), (1, nc.gpsimd)):
                    p0 = 64 * he
                    eps = pp1.tile([128, 512], F32, tag="ps1", name="eps")
                    mm(eps[:], qec[p0:p0 + 64, 0:128],
                       kt[p0:p0 + 64, c * 512:(c + 1) * 512])
                    ee = eep.tile([128, 512], BF16, tag="ee", name="ee")
                    nc.vector.tensor_copy(ee[:], eps[:])
                    h = 2 * pair + he
                    dma_eng.dma_start(
                        out=pe1[h * 128:(h + 1) * 128,
                                c * 512:(c + 1) * 512], in_=ee[:])

            def e2_unit(pair, qec):
                kt = kts[pair]
                for he in (0, 1):
                    p0 = 64 * he
                    e2ps = pp1.tile([128, 6 * BLK], F32, tag="ps1",
                                    name="e2ps")
                    mm(e2ps[:, 0:192], qec[p0:p0 + 64, 128:256],
                       kt[p0:p0 + 64, 0:192])
                    mm(e2ps[:, 192:384], qec[p0:p0 + 64, 128:256],
                       kt[p0:p0 + 64, S - 192:S])
                    e2ev = eep.tile([128, 6 * BLK], BF16, tag="e2ev",
                                    name="e2ev")
                    nc.vector.tensor_copy(e2ev[:], e2ps[:])
                    h = 2 * pair + he
                    nc.sync.dma_start(out=pe2[h * 128:(h + 1) * 128, :],
                                      in_=e2ev[:])

            # phase blocks. Per slot the PE emission order is
            #   [QK(u) 64-row mode][e1 64-mode][proj 128-mode][PV(u-1) 128]
            # so the lagged PV never waits on exp, and same-mode matmuls
            # stay batched (a tiling-mode change drains the PE array).
            pair_tiles = {0: make_pair_tiles(0)}

            def mk_pair(p):
                def f():
                    pair_tiles[p] = make_pair_tiles(p)
                return f

            pv_lag = []

            def phase(couples, fill64, fill128):
                # couple (pair, u0) = units u0, u0+1 of one pair per slot:
                # [QK QK][e1 64-mode][proj 128-mode][PV PV of prev couple]
                # keeps same-tiling-mode matmuls batched (mode change =
                # array drain) and the lagged PV clear of its exp
                k64, k128 = 0, 0
                n_slots = len(couples)
                for i, (pair, u0) in enumerate(couples):
                    pts = [(u, unit_qk(pair, u, pair_tiles[pair][0]))
                           for u in (u0, u0 + 1)]
                    pv_lag.append((pair, pts))
                    want = (i + 1) * len(fill64) // n_slots
                    while k64 < want:
                        fill64[k64]()
                        k64 += 1
                    want = (i + 1) * len(fill128) // n_slots
                    while k128 < want:
                        fill128[k128]()
                        k128 += 1
                    while len(pv_lag) > 1:
                        unit_pv(*pv_lag.pop(0))

            def e1_fills(pair):
                qec = pair_tiles[pair][1]
                out = [lambda c=c: e1_unit(pair, c, qec)
                       for c in range(NSEQ)]
                out.append(lambda: e2_unit(pair, qec))
                return out

            # phase A: pair 0; its edges as 64-mode filler; ALL of
            # k1/q1/k2/q2 (+ pair 1/2 kg/qec builds) as 128-mode filler.
            # Front-loading proj leaves the later phases balanced.
            phase([(0, u0) for u0 in range(0, NU, 2)], e1_fills(0),
                  kq_groups(1) + [mk_pair(1)] + kq_groups(2) + [mk_pair(2)])
            # phase BC head: pairs 1, 2 alternating couples; k3/q3 fillers
            units_bc = []
            for u0 in range(0, 14, 2):
                units_bc += [(1, u0), (2, u0)]
            phase(units_bc, [], kq_groups(3) + [mk_pair(3)])
            # phase BCD tail: remaining B/C couples merged with all of
            # pair 3 (kt3/qt3 completed above) plus all remaining edge
            # work, so no stretch runs filler-starved
            qb = [(1, u0) for u0 in range(14, NU, 2)]
            qc = [(2, u0) for u0 in range(14, NU, 2)]
            qd = [(3, u0) for u0 in range(0, NU, 2)]
            tail = []
            while qb or qc or qd:
                for src in (qb, qd, qc, qd):
                    if src:
                        tail.append(src.pop(0))
            phase(tail, e1_fills(1) + e1_fills(2) + e1_fills(3), [])
            while pv_lag:
                unit_pv(*pv_lag.pop(0))
    nc.compile()
    return nc


def _wshuf(W, fs, scale=1.0):
    wt = np.asarray(W, np.float32)[fs, :].T * scale  # [HS, FPC]
    return np.ascontiguousarray(
        wt.reshape(NKC, 128, FPC).transpose(1, 0, 2)).astype(NPBF16)


def _host_inputs(hidden, Wq, bq, Wk, bk, Wv, bv, c):
    b, hh = c // 2, c % 2
    fs = slice(hh * FPC, (hh + 1) * FPC)
    X = np.asarray(hidden[b], np.float32)
    xt = np.ascontiguousarray(
        X.reshape(NSEQ, 512, NKC, 128).transpose(3, 0, 2, 1)).astype(NPBF16)
    return {
        "xt": xt,
        "wq": _wshuf(Wq, fs, 0.125),
        "wk": _wshuf(Wk, fs),
        "wv": _wshuf(Wv, fs),
        "bqs": np.ascontiguousarray(
            (bq[fs].astype(np.float32) * 0.125).reshape(4, 128).T),
        "bks": np.ascontiguousarray(
            bk[fs].astype(np.float32).reshape(4, 128).T),
    }


def _host_finish(res_c, v, bvh):
    """Per-core host post-processing -> [S, FPC] output slice.
    v: host-computed v WITH bias [S, FPC]; bvh: bv slice [FPC]."""
    ctxt = np.asarray(res_c["ctxt"], np.float32).reshape(4, 65, 2, NU * 128)
    p1 = np.asarray(res_c["pe1"], np.float32)
    p2 = np.asarray(res_c["pe2"], np.float32)
    out = np.empty((S, FPC), np.float32)
    for h in range(HPC):
        pair, he = h // 2, h % 2
        vh = v[:, h * 64:(h + 1) * 64]
        # middle blocks 2..61 (device v had no bias; sum(w)=1 -> add bv)
        num = ctxt[pair, 0:64, he]
        den = ctxt[pair, 64, he]
        out[2 * BLK:62 * BLK, h * 64:(h + 1) * 64] = \
            (num / den).T + bvh[h * 64:(h + 1) * 64][None, :]
        # E1: blocks 0, 63 (full attention); device ships raw scores
        P = np.exp(p1[h * 128:(h + 1) * 128, :])
        C = (P / P.sum(1, keepdims=True)) @ vh
        out[0:BLK, h * 64:(h + 1) * 64] = C[0:64]
        out[S - BLK:S, h * 64:(h + 1) * 64] = C[64:128]
        # E2: blocks 1, 62; key cols = blocks {0,1,2} then {61,62,63}
        P = np.exp(p2[h * 128:(h + 1) * 128, :])
        P[0:64, 192:320] = 0.0    # block 1 bans blocks 61, 62
        P[64:128, 64:192] = 0.0   # block 62 bans blocks 1, 2
        vk = np.concatenate([vh[0:192], vh[(NB - 3) * BLK:]], 0)
        C = (P / P.sum(1, keepdims=True)) @ vk
        out[BLK:2 * BLK, h * 64:(h + 1) * 64] = C[0:64]
        out[62 * BLK:63 * BLK, h * 64:(h + 1) * 64] = C[64:128]
    return out


def _run(inputs, trace=False):
    global _BUILT
    if _BUILT is None:
        _BUILT = _build()
    core_ids = list(range(8))
    in_maps = [_host_inputs(**inputs, c=c) for c in core_ids]
    res = run_bass_kernel_spmd(_BUILT, in_maps, core_ids, trace=trace)
    out = np.empty((B, S, HS), np.float32)
    Wv = np.asarray(inputs["Wv"], np.float32)
    bv = np.asarray(inputs["bv"], np.float32)
    for c in core_ids:
        b, hh = c // 2, c % 2
        fs = slice(hh * FPC, (hh + 1) * FPC)
        X16 = np.asarray(inputs["hidden"][b]).astype(NPBF16)
        W16 = Wv[fs, :].astype(NPBF16)
        bvh = bv[fs].astype(NPBF16).astype(np.float32)
        v = (X16.astype(np.float32) @ W16.astype(np.float32).T
             + bvh).astype(NPBF16)
        out[b, :, hh * FPC:(hh + 1) * FPC] = _host_finish(
            res.results[c], v.astype(np.float32), bvh)
    return out, res


def kernel(hidden_states, Wq, bq, Wk, bk, Wv, bv):
    inputs = dict(hidden=np.asarray(hidden_states), Wq=np.asarray(Wq),
                  bq=np.asarray(bq), Wk=np.asarray(Wk),
                  bk=np.asarray(bk), Wv=np.asarray(Wv), bv=np.asarray(bv))
    out, _ = _run(inputs, trace=False)
    return out


# revision 29
# speedup vs baseline: 1.0068x; 1.0068x over previous
"""BigBird block-sparse attention TRN2 kernel v2 (8 NeuronCores, SPMD).

Sharding: core c handles batch b=c//2 and head-half hh=c%2 (8 of 16 heads,
feature slice hh*512..+512). All matmul I/O in bf16 (fp32 PSUM accumulate).

v2 structure (vs v1): heads processed in PAIRS with the even head's K=64
matmuls on PE rows 0-63 and the odd head's on rows 64-127, emitted
adjacently so the row-tiled matmuls run concurrently (~2x on QK^T and the
edge scores). Middle blocks are processed in 30 half-strip units per pair
(2 query blocks x 2 heads), with sps PSUM laid out bank-disjoint between
the heads ([128,7,128]: even groups 0-2 in bank 0, odd 4-6 in bank 1).
exp runs as one ACT instruction over a strided 2x384 AP; sliding-window
bans are GpSimd memsets on the bf16 pt tile. Projection m-tile passes and
edge-block scores are interleaved into the unit stream as PE filler so the
PE never idles (HAM stays at K=8/8). q-scale (1/8) and q-bias are folded
into Wq/bq host-side; v carries no bias on device (host adds bv after
normalization since sum(softmax)=1).

Per core, single pass, q/k/v SBUF-resident:
  1. v-pass (streams x once): v = X@Wv.T -> via SBUF->SBUF shift DMAs into
     vsh [128, 32, 520]: 64-row-shifted key chunks (chunk c = seq 64+128c),
     chunk 31 = [block63 | block0], ones column per head (col h*65+64) for
     softmax denominators.
  2. k/q m-tile passes: k.T, q.T feature-major [128, 4096] bf16 tiles,
     biases via DVE tensor_scalar.
  3. per pair: 30 half-units (QK^T paired-row matmuls -> exp -> ban ->
     PV vs vsh chunks + denominator rider), edge blocks 0/63 raw scores
     vs all keys and 1/62 vs 6 key blocks shipped to host.
Host: normalizes middle ctx (+bv), computes edge softmax+PV, reassembles.
"""
import sys

if "/opt/trn_rl_repo" not in sys.path:
    sys.path.insert(0, "/opt/trn_rl_repo")

import numpy as np
import ml_dtypes

import concourse.bacc as bacc
import concourse.bass as bass
import concourse.tile as tile
from concourse import mybir
from concourse.bass_utils import run_bass_kernel_spmd

F32 = mybir.dt.float32
BF16 = mybir.dt.bfloat16
NPBF16 = ml_dtypes.bfloat16

B, S, H, HS, D, BLK = 4, 4096, 16, 1024, 64, 64
NB = S // BLK            # 64 key/query blocks
HPC = 8                  # heads per core
FPC = HPC * D            # 512 features per core
NKC = HS // 128          # 8 contraction chunks
NSEQ = 8                 # seq chunks of 512
NU = 30                  # half-strip units per pair (q blocks 2..61)
GC = 31                  # [blk63|blk0] global chunk slot in vsh

_BUILT = None


def _build():
    nc = bacc.Bacc(None, target_bir_lowering=False)

    # ---- parameters ----
    # xt[p, n, kc, s'] = X[n*512+s', kc*128+p]
    xt = nc.declare_dram_parameter("xt", [128, NSEQ, NKC, 512], BF16, False)
    # w*[p, kc, f] = W.T[kc*128+p, f]  (feature slice; wq pre-scaled by 1/8)
    wq = nc.declare_dram_parameter("wq", [128, NKC, FPC], BF16, False)
    wk = nc.declare_dram_parameter("wk", [128, NKC, FPC], BF16, False)
    wv = nc.declare_dram_parameter("wv", [128, NKC, FPC], BF16, False)
    bqs = nc.declare_dram_parameter("bqs", [128, 4], F32, False)  # /8 applied
    bks = nc.declare_dram_parameter("bks", [128, 4], F32, False)

    # ctxt[pair*65+r, he*3840 + u*128 + q] : r<64 numerator, r=64 denominator
    ctxt = nc.declare_dram_parameter("ctxt", [4 * 65, 2 * NU * 128], BF16, True)
    pe1 = nc.declare_dram_parameter("pe1", [HPC * 128, S], BF16, True)
    pe2 = nc.declare_dram_parameter("pe2", [HPC * 128, 6 * BLK], BF16, True)

    with tile.TileContext(nc) as tc:
        with tc.tile_pool(name="const", bufs=1) as cp, \
             tc.tile_pool(name="big", bufs=1) as bp, \
             tc.tile_pool(name="x", bufs=1) as xp, \
             tc.tile_pool(name="evac", bufs=2) as ep, \
             tc.tile_pool(name="pair", bufs=3) as prp, \
             tc.tile_pool(name="pt", bufs=4) as ptp, \
             tc.tile_pool(name="ctx", bufs=3) as cxp, \
             tc.tile_pool(name="ee", bufs=4) as eep, \
             tc.tile_pool(name="ps1", bufs=2, space="PSUM") as pp1, \
             tc.tile_pool(name="qk", bufs=2, space="PSUM") as qkp, \
             tc.tile_pool(name="sm", bufs=2, space="PSUM") as smp:

            # ---- input DMAs, ordered so arrival matches the init phase's
            # consumption: wv gates the first matmul; xt_n stream in around
            # the per-n group rate; wk/wq needed from the first k0/q0 group
            wvt = bp.tile([128, NKC, FPC], BF16, tag="wv_kt3", name="wvt")
            xts = [xp.tile([128, NKC, 512], BF16, tag=f"xt{n}", name=f"xt{n}")
                   for n in range(NSEQ)]
            wts = {"v": wvt}
            for name, w in (("k", wk), ("q", wq)):
                wts[name] = cp.tile([128, NKC, FPC], BF16, tag=f"w{name}",
                                    name=f"w{name}t")
            bqt = cp.tile([128, 4], F32, tag="bqt")
            bkt = cp.tile([128, 4], F32, tag="bkt")
            # first v matmul gates on wv + xt0 only: those two lead their
            # queues; later xt chunks stream in around consumption order
            nc.sync.dma_start(out=xts[0][:], in_=xt[:, 0])
            nc.scalar.dma_start(out=wvt[:], in_=wv[:])
            nc.gpsimd.dma_start(out=wts["k"][:], in_=wk[:])
            nc.sync.dma_start(out=xts[1][:], in_=xt[:, 1])
            nc.gpsimd.dma_start(out=wts["q"][:], in_=wq[:])
            nc.sync.dma_start(out=bqt[:], in_=bqs[:])
            nc.sync.dma_start(out=bkt[:], in_=bks[:])
            nc.sync.dma_start(out=xts[2][:], in_=xt[:, 2])
            for n, eng in ((3, nc.scalar), (4, nc.sync), (5, nc.scalar),
                           (6, nc.gpsimd), (7, nc.sync)):
                eng.dma_start(out=xts[n][:], in_=xt[:, n])

            # vsh: shifted v chunks + ones cols. [128, 32, 520] bf16
            vsh = bp.tile([128, 32, 520], BF16, tag="vsh")
            ones_base = vsh[:, :, 0:1]
            nc.vector.memset(
                bass.AP(tensor=ones_base.tensor, offset=ones_base.offset + 64,
                        ap=[ones_base.ap[0], [520, 32], [65, HPC], [1, 1]]),
                1.0,
            )

            # ---- v-pass ----
            ev4 = None

            def shift_dma(pdst, psrc, c0, sm0, nch):
                dst = vsh[pdst:pdst + 64, c0, 0:64]
                src = ev4[psrc:psrc + 64, sm0, 0:64]
                nc.scalar.dma_start(
                    out=bass.AP(tensor=dst.tensor, offset=dst.offset,
                                ap=[dst.ap[0], [520, nch], [65, HPC], [1, 64]]),
                    in_=bass.AP(tensor=src.tensor, offset=src.offset,
                                ap=[src.ap[0], [512, nch], [64, HPC], [1, 64]]),
                )

            def v_group(g):
                nonlocal ev4
                n, half = g // 2, g % 2
                ev4 = ep.tile([128, 2, 512], BF16, tag="ev4", name="ev4")
                for sm2 in range(2):
                    sm = 2 * half + sm2
                    ps = pp1.tile([128, 512], F32, tag="ps1")
                    for kc in range(NKC):
                        nc.tensor.matmul(
                            ps[:],
                            xts[n][:, kc, sm * 128:(sm + 1) * 128],
                            wts["v"][:, kc, :],
                            start=(kc == 0), stop=(kc == NKC - 1),
                        )
                    nc.vector.tensor_copy(ev4[:, sm2, :], ps[:])
                if g == 0:
                    shift_dma(64, 0, GC, 0, 1)
                    shift_dma(64, 0, 0, 1, 1)
                else:
                    shift_dma(64, 0, 2 * g - 1, 0, 2)
                shift_dma(0, 64, 2 * g, 0, 2)

            # ---- k/q m-tile pass groups (interleaved as PE filler) ----
            kts, qts = {}, {}

            def kq_group(name, store, bt, mt, n):
                if mt not in store:
                    tag = "wv_kt3" if (name == "k" and mt == 3) else \
                        f"{name}t{mt}"
                    store[mt] = bp.tile([128, S], BF16, tag=tag,
                                        name=f"{name}t{mt}")
                dst = store[mt]
                ps = pp1.tile([128, 512], F32, tag="ps1", name="ps")
                for kc in range(NKC):
                    nc.tensor.matmul(
                        ps[:],
                        wts[name][:, kc, mt * 128:(mt + 1) * 128],
                        xts[n][:, kc, :],
                        start=(kc == 0), stop=(kc == NKC - 1),
                    )
                nc.vector.tensor_scalar(
                    out=dst[:, n * 512:(n + 1) * 512], in0=ps[:],
                    scalar1=bt[:, mt:mt + 1], scalar2=None,
                    op0=mybir.AluOpType.add,
                )

            def kq_groups(mt):
                out = []
                for name, store, bt in (("k", kts, bkt), ("q", qts, bqt)):
                    for n in range(NSEQ):
                        out.append(lambda name=name, store=store, bt=bt,
                                   n=n: kq_group(name, store, bt, mt, n))
                return out

            # init: v-pass with k0/q0 interleaved per x-chunk so compute
            # consumption matches the xt DMA delivery rate (no PE stall)
            for n in range(NSEQ):
                v_group(2 * n)
                v_group(2 * n + 1)
                kq_group("k", kts, bkt, 0, n)
                kq_group("q", qts, bqt, 0, n)

            # ---- pair processing ----
            def mm(out, lhsT, rhs, start=True, stop=True):
                nc.tensor.matmul(out, lhsT, rhs, start=start, stop=stop)

            EXP = mybir.ActivationFunctionType.Exp

            def make_pair_tiles(pair):
                kt, qt = kts[pair], qts[pair]
                kg = prp.tile([128, 128], BF16, tag="kg")
                qec = prp.tile([128, 256], BF16, tag="qec")
                for p0 in (0, 64):
                    nc.vector.tensor_copy(kg[p0:p0 + 64, 0:64],
                                          kt[p0:p0 + 64, S - 64:S])
                    nc.vector.tensor_copy(kg[p0:p0 + 64, 64:128],
                                          kt[p0:p0 + 64, 0:64])
                    nc.vector.tensor_copy(qec[p0:p0 + 64, 0:64],
                                          qt[p0:p0 + 64, 0:64])
                    nc.vector.tensor_copy(qec[p0:p0 + 64, 64:128],
                                          qt[p0:p0 + 64, S - 64:S])
                    nc.vector.tensor_copy(qec[p0:p0 + 64, 128:192],
                                          qt[p0:p0 + 64, 64:128])
                    nc.vector.tensor_copy(qec[p0:p0 + 64, 192:256],
                                          qt[p0:p0 + 64, S - 128:S - 64])
                return kg, qec

            ctx_cur = {}

            def unit_qk(pair, u, kg):
                """Half-strip QK^T: q blocks 2+2u, 3+2u; paired rows.
                sps [128,7,128]: even head groups 0-2 (bank 0), odd 4-6
                (bank 1); adjacent e/o emission -> concurrent row tiles."""
                kt, qt = kts[pair], qts[pair]
                q0 = (2 + 2 * u) * BLK
                ca = (2 * u + 1) * BLK
                sps = qkp.tile([128, 7, 128], F32, tag="sps", name="sps")
                qa = {0: qt[0:64, q0:q0 + 128], 64: qt[64:128, q0:q0 + 128]}
                mm(sps[:, 0, :], kg[0:64, :], qa[0])
                mm(sps[:, 4, :], kg[64:128, :], qa[64])
                mm(sps[:, 1, :], kt[0:64, ca:ca + 128], qa[0])
                mm(sps[:, 5, :], kt[64:128, ca:ca + 128], qa[64])
                mm(sps[:, 2, :], kt[0:64, ca + 128:ca + 256], qa[0])
                mm(sps[:, 6, :], kt[64:128, ca + 128:ca + 256], qa[64])
                pt = ptp.tile([128, 7, 128], BF16, tag="pt", name="pt")
                # one exp over both heads' 384-col blocks via strided AP
                src = sps[:, 0, :]
                dst = pt[:, 0, :]
                nc.scalar.activation(
                    bass.AP(tensor=dst.tensor, offset=dst.offset,
                            ap=[dst.ap[0], [512, 2], [1, 384]]),
                    bass.AP(tensor=src.tensor, offset=src.offset,
                            ap=[src.ap[0], [512, 2], [1, 384]]),
                    EXP,
                )
                # ban invalid sliding quadrants: {g1,g5}, {g2,g6}
                # (GpSimd: pt is SBUF-only, frees DVE for evac copies)
                lo = pt[0:64, 1, 64:128]
                nc.gpsimd.memset(
                    bass.AP(tensor=lo.tensor, offset=lo.offset,
                            ap=[lo.ap[0], [512, 2], [1, 64]]), 0.0)
                hi = pt[64:128, 2, 0:64]
                nc.gpsimd.memset(
                    bass.AP(tensor=hi.tensor, offset=hi.offset,
                            ap=[hi.ap[0], [512, 2], [1, 64]]), 0.0)
                return pt

            def unit_pv(pair, items):
                """PV for a couple of same-pair units into one cps bank;
                one evac copy for both."""
                cps = smp.tile([65, 2, 256], F32, tag="cps", name="cps")
                for j, (u, pt) in enumerate(items):
                    for he in (0, 1):
                        hc = (2 * pair + he) * 65
                        g0 = 4 * he
                        sl = cps[:, he, j * 128:(j + 1) * 128]
                        mm(sl, vsh[:, GC, hc:hc + 65], pt[:, g0, :],
                           start=True, stop=False)
                        mm(sl, vsh[:, u, hc:hc + 65], pt[:, g0 + 1, :],
                           start=False, stop=False)
                        mm(sl, vsh[:, u + 1, hc:hc + 65], pt[:, g0 + 2, :],
                           start=False, stop=True)
                u0 = items[0][0]
                seg, off = u0 // 6, u0 % 6
                if off == 0:
                    ctx_cur[pair] = cxp.tile([65, 2, 6 * 128], BF16,
                                             tag="ctx", name="ctx")
                ctx = ctx_cur[pair]
                w = 128 * len(items)
                nc.vector.tensor_copy(ctx[:, :, off * 128:off * 128 + w],
                                      cps[:, :, 0:w])
                if off + len(items) == 6:
                    dst = ctxt[pair * 65:(pair + 1) * 65,
                               seg * 768:seg * 768 + 768]
                    nc.sync.dma_start(
                        out=bass.AP(tensor=dst.tensor, offset=dst.offset,
                                    ap=[dst.ap[0], [NU * 128, 2], [1, 768]]),
                        in_=ctx[:],
                    )

            def e1_unit(pair, c, qec):
                kt = kts[pair]
                for he, dma_eng in ((0, nc.sync), (1, nc.gpsimd)):
                    p0 = 64 * he
                    eps = pp1.tile([128, 512], F32, tag="ps1", name="eps")
                    mm(eps[:], qec[p0:p0 + 64, 0:128],
                       kt[p0:p0 + 64, c * 512:(c + 1) * 512])
                    ee = eep.tile([128, 512], BF16, tag="ee", name="ee")
                    nc.vector.tensor_copy(ee[:], eps[:])
                    h = 2 * pair + he
                    dma_eng.dma_start(
                        out=pe1[h * 128:(h + 1) * 128,
                                c * 512:(c + 1) * 512], in_=ee[:])

            def e2_unit(pair, qec):
                kt = kts[pair]
                for he in (0, 1):
                    p0 = 64 * he
                    e2ps = pp1.tile([128, 6 * BLK], F32, tag="ps1",
                                    name="e2ps")
                    mm(e2ps[:, 0:192], qec[p0:p0 + 64, 128:256],
                       kt[p0:p0 + 64, 0:192])
                    mm(e2ps[:, 192:384], qec[p0:p0 + 64, 128:256],
                       kt[p0:p0 + 64, S - 192:S])
                    e2ev = eep.tile([128, 6 * BLK], BF16, tag="e2ev",
                                    name="e2ev")
                    nc.vector.tensor_copy(e2ev[:], e2ps[:])
                    h = 2 * pair + he
                    nc.sync.dma_start(out=pe2[h * 128:(h + 1) * 128, :],
                                      in_=e2ev[:])

            # phase blocks. Per slot the PE emission order is
            #   [QK(u) 64-row mode][e1 64-mode][proj 128-mode][PV(u-1) 128]
            # so the lagged PV never waits on exp, and same-mode matmuls
            # stay batched (a tiling-mode change drains the PE array).
            pair_tiles = {0: make_pair_tiles(0)}

            def mk_pair(p):
                def f():
                    pair_tiles[p] = make_pair_tiles(p)
                return f

            pv_lag = []

            def phase(couples, fill64, fill128):
                # couple (pair, u0) = units u0, u0+1 of one pair per slot:
                # [QK QK][e1 64-mode][proj 128-mode][PV PV of prev couple]
                # keeps same-tiling-mode matmuls batched (mode change =
                # array drain) and the lagged PV clear of its exp
                k64, k128 = 0, 0
                n_slots = len(couples)
                for i, (pair, u0) in enumerate(couples):
                    pts = [(u, unit_qk(pair, u, pair_tiles[pair][0]))
                           for u in (u0, u0 + 1)]
                    pv_lag.append((pair, pts))
                    want = (i + 1) * len(fill64) // n_slots
                    while k64 < want:
                        fill64[k64]()
                        k64 += 1
                    # emit 128-mode fillers in adjacent pairs so only one
                    # tiling-mode switch is paid per two proj groups
                    want = (i + 1) * len(fill128) // n_slots
                    if want - k128 == 1 and k128 + 2 <= len(fill128) and \
                            (i + 2) * len(fill128) // n_slots > want:
                        want = k128 + 2
                    while k128 < want:
                        fill128[k128]()
                        k128 += 1
                    while len(pv_lag) > 1:
                        unit_pv(*pv_lag.pop(0))

            def e1_fills(pair):
                qec = pair_tiles[pair][1]
                out = [lambda c=c: e1_unit(pair, c, qec)
                       for c in range(NSEQ)]
                out.append(lambda: e2_unit(pair, qec))
                return out

            # phase A: pair 0; its edges as 64-mode filler; ALL of
            # k1/q1/k2/q2 (+ pair 1/2 kg/qec builds) as 128-mode filler.
            # Front-loading proj leaves the later phases balanced.
            phase([(0, u0) for u0 in range(0, NU, 2)], e1_fills(0),
                  kq_groups(1) + [mk_pair(1)] + kq_groups(2) + [mk_pair(2)])
            # phase BC head: pairs 1, 2 alternating couples; k3/q3 fillers
            units_bc = []
            for u0 in range(0, 14, 2):
                units_bc += [(1, u0), (2, u0)]
            phase(units_bc, [], kq_groups(3) + [mk_pair(3)])
            # phase BCD tail: remaining B/C couples merged with all of
            # pair 3 (kt3/qt3 completed above) plus all remaining edge
            # work, so no stretch runs filler-starved
            qb = [(1, u0) for u0 in range(14, NU, 2)]
            qc = [(2, u0) for u0 in range(14, NU, 2)]
            qd = [(3, u0) for u0 in range(0, NU, 2)]
            tail = []
            while qb or qc or qd:
                for src in (qb, qd, qc, qd):
                    if src:
                        tail.append(src.pop(0))
            phase(tail, e1_fills(1) + e1_fills(2) + e1_fills(3), [])
            while pv_lag:
                unit_pv(*pv_lag.pop(0))
    nc.compile()
    return nc


def _wshuf(W, fs, scale=1.0):
    wt = np.asarray(W, np.float32)[fs, :].T * scale  # [HS, FPC]
    return np.ascontiguousarray(
        wt.reshape(NKC, 128, FPC).transpose(1, 0, 2)).astype(NPBF16)


def _host_inputs(hidden, Wq, bq, Wk, bk, Wv, bv, c):
    b, hh = c // 2, c % 2
    fs = slice(hh * FPC, (hh + 1) * FPC)
    X = np.asarray(hidden[b], np.float32)
    xt = np.ascontiguousarray(
        X.reshape(NSEQ, 512, NKC, 128).transpose(3, 0, 2, 1)).astype(NPBF16)
    return {
        "xt": xt,
        "wq": _wshuf(Wq, fs, 0.125),
        "wk": _wshuf(Wk, fs),
        "wv": _wshuf(Wv, fs),
        "bqs": np.ascontiguousarray(
            (bq[fs].astype(np.float32) * 0.125).reshape(4, 128).T),
        "bks": np.ascontiguousarray(
            bk[fs].astype(np.float32).reshape(4, 128).T),
    }


def _host_finish(res_c, v, bvh):
    """Per-core host post-processing -> [S, FPC] output slice.
    v: host-computed v WITH bias [S, FPC]; bvh: bv slice [FPC]."""
    ctxt = np.asarray(res_c["ctxt"], np.float32).reshape(4, 65, 2, NU * 128)
    p1 = np.asarray(res_c["pe1"], np.float32)
    p2 = np.asarray(res_c["pe2"], np.float32)
    out = np.empty((S, FPC), np.float32)
    for h in range(HPC):
        pair, he = h // 2, h % 2
        vh = v[:, h * 64:(h + 1) * 64]
        # middle blocks 2..61 (device v had no bias; sum(w)=1 -> add bv)
        num = ctxt[pair, 0:64, he]
        den = ctxt[pair, 64, he]
        out[2 * BLK:62 * BLK, h * 64:(h + 1) * 64] = \
            (num / den).T + bvh[h * 64:(h + 1) * 64][None, :]
        # E1: blocks 0, 63 (full attention); device ships raw scores
        P = np.exp(p1[h * 128:(h + 1) * 128, :])
        C = (P / P.sum(1, keepdims=True)) @ vh
        out[0:BLK, h * 64:(h + 1) * 64] = C[0:64]
        out[S - BLK:S, h * 64:(h + 1) * 64] = C[64:128]
        # E2: blocks 1, 62; key cols = blocks {0,1,2} then {61,62,63}
        P = np.exp(p2[h * 128:(h + 1) * 128, :])
        P[0:64, 192:320] = 0.0    # block 1 bans blocks 61, 62
        P[64:128, 64:192] = 0.0   # block 62 bans blocks 1, 2
        vk = np.concatenate([vh[0:192], vh[(NB - 3) * BLK:]], 0)
        C = (P / P.sum(1, keepdims=True)) @ vk
        out[BLK:2 * BLK, h * 64:(h + 1) * 64] = C[0:64]
        out[62 * BLK:63 * BLK, h * 64:(h + 1) * 64] = C[64:128]
    return out


def _run(inputs, trace=False):
    global _BUILT
    if _BUILT is None:
        _BUILT = _build()
    core_ids = list(range(8))
    in_maps = [_host_inputs(**inputs, c=c) for c in core_ids]
    res = run_bass_kernel_spmd(_BUILT, in_maps, core_ids, trace=trace)
    out = np.empty((B, S, HS), np.float32)
    Wv = np.asarray(inputs["Wv"], np.float32)
    bv = np.asarray(inputs["bv"], np.float32)
    for c in core_ids:
        b, hh = c // 2, c % 2
        fs = slice(hh * FPC, (hh + 1) * FPC)
        X16 = np.asarray(inputs["hidden"][b]).astype(NPBF16)
        W16 = Wv[fs, :].astype(NPBF16)
        bvh = bv[fs].astype(NPBF16).astype(np.float32)
        v = (X16.astype(np.float32) @ W16.astype(np.float32).T
             + bvh).astype(NPBF16)
        out[b, :, hh * FPC:(hh + 1) * FPC] = _host_finish(
            res.results[c], v.astype(np.float32), bvh)
    return out, res


def kernel(hidden_states, Wq, bq, Wk, bk, Wv, bv):
    inputs = dict(hidden=np.asarray(hidden_states), Wq=np.asarray(Wq),
                  bq=np.asarray(bq), Wk=np.asarray(Wk),
                  bk=np.asarray(bk), Wv=np.asarray(Wv), bv=np.asarray(bv))
    out, _ = _run(inputs, trace=False)
    return out
